# revision 4
# baseline (speedup 1.0000x reference)
"""Multi-head attention Trainium2 Bass kernel.

Problem: B=2, S=2048, D=1024, H=16, HS=64.
Sharding: tensor-parallel over heads — each of 8 cores computes 2 heads
(128 contiguous output-feature columns) for both batches; host concatenates.

Per-core pipeline (v2 — restructured around the measured bottlenecks:
the exp stream on the Activation engine, ~1114 ns per [128,1024] tile,
is the steady-state limiter; projections and the initial X^T DMA were
serial prologue in v1):
  1. X^T (bf16, host pre-transposed) DMAd in (t-tile, chunk) slice order so
     the first projection starts after ~1/8 of the transfer. Weights ride
     the scalar/gpsimd DMA queues concurrently.
  2. Projections in bf16 (psum fp32): Qt/Kt feature-major [128 feat, tok];
     the bias add is fused into the PSUM->SBUF copy on DVE (tensor_scalar
     with a per-partition bias column) instead of a K=1 matmul. V' is
     token-major per 128-token chunk with layout [V_h0 | 1 | V_h1 | 1]
     (ones columns produce the softmax denominator as output row 65 of PV).
  3. Attention runs one (batch, q-half, head) at a time: 16 k-chunk steps of
     sim^T = K-chunk^T Q (K=64 matmuls at the head's partition offset),
     P^T = exp(sim^T/8) via one [128,1024] ACT per step, O'^T += V'^T P^T
     accumulated over the k loop in PSUM. PSUM budget: sims 2x2 banks
     (double buffered) + O' 2 banks + 2 banks for interleaved projections.
  4. Batch-1 projections (and b0's second-q-half Q tiles) are emitted
     inside the attention k-loops, paced ~1 matmul per step, to run in the
     Tensor engine's slack while ACT streams exp.
  5. Unnormalized O'^T (65 rows: 64 features + denominator) goes to DRAM;
     the host divides and transposes during assembly.
"""

import sys

sys.path.insert(0, "/opt/trn_rl_repo")

import ml_dtypes
import numpy as np

import concourse.bass as bass
import concourse.mybir as mybir
import concourse.tile as tile
from concourse import bacc
from concourse import bass_utils

B, S, D = 2, 2048, 1024
H, HS = 16, 64
NCORES = 8
NTOK = B * S                  # 4096
FPC = (H // NCORES) * HS      # 128 output-feature cols per core (2 heads)
TT = 512                      # token tile for projections
NTT = NTOK // TT              # 8
NCH = D // 128                # 8 contraction chunks
QT = 512                      # q tile (one matmul / psum bank)
QH = 2 * QT                   # 1024-wide q half
KT = 128                      # k chunk in attention
NKT = S // KT                 # 16
VW = 2 * (HS + 1)             # 130: [V_h0 | 1 | V_h1 | 1] columns

F32 = mybir.dt.float32
BF16 = mybir.dt.bfloat16

_NC_CACHE = {}


def build_nc():
    nc = bacc.Bacc("TRN2", target_bir_lowering=False, debug=False, num_devices=NCORES)
    xt = nc.dram_tensor("xt", [D, NTOK], BF16, kind="ExternalInput").ap()
    wq = nc.dram_tensor("wq", [D, FPC], F32, kind="ExternalInput").ap()
    wk = nc.dram_tensor("wk", [D, FPC], F32, kind="ExternalInput").ap()
    wvp = nc.dram_tensor("wvp", [D, VW], F32, kind="ExternalInput").ap()
    bq = nc.dram_tensor("bq", [FPC, 1], F32, kind="ExternalInput").ap()
    bk = nc.dram_tensor("bk", [FPC, 1], F32, kind="ExternalInput").ap()
    bvp = nc.dram_tensor("bvp", [1, VW], F32, kind="ExternalInput").ap()
    ones = nc.dram_tensor("ones", [1, KT], F32, kind="ExternalInput").ap()
    out = nc.dram_tensor("out", [2 * (HS + 1), NTOK], F32, kind="ExternalOutput").ap()

    with tile.TileContext(nc) as tc:
        with (
            tc.tile_pool(name="persist", bufs=1) as pp,
            tc.tile_pool(name="work", bufs=2) as wk_pool,
            tc.tile_pool(name="psA", bufs=2, space="PSUM") as psA,   # sims: 2x2 banks
            tc.tile_pool(name="psB", bufs=1, space="PSUM") as psB,   # O' accum: 2 banks
            tc.tile_pool(name="psP", bufs=2, space="PSUM") as psP,   # proj: 2x1 bank
        ):
            # ---------------- init: weights, X^T ------------------------------
            wq_st = pp.tile([128, NCH * FPC], F32)
            wk_st = pp.tile([128, NCH * FPC], F32)
            wv_st = pp.tile([128, NCH * VW], F32)
            xtc = [pp.tile([128, NTOK], BF16, name=f"xt_{c}") for c in range(NCH)]
            wq_b = pp.tile([128, NCH * FPC], BF16)
            wk_b = pp.tile([128, NCH * FPC], BF16)
            wv_b = pp.tile([128, NCH * VW], BF16)
            bq_sb = pp.tile([128, 1], F32)
            bk_sb = pp.tile([128, 1], F32)
            rows_st = pp.tile([1, VW + KT], F32)
            rows_b = pp.tile([1, VW + KT], BF16)

            # K weights first on the scalar HWDGE queue (idle pre-attention),
            # so the first projection's weights beat its activations.
            for c in range(NCH):
                nc.scalar.dma_start(wk_st[:, c * FPC : (c + 1) * FPC], wk[c * 128 : (c + 1) * 128, :])
            nc.vector.tensor_copy(wk_b[:], wk_st[:])
            for c in range(NCH):
                nc.gpsimd.dma_start(wv_st[:, c * VW : (c + 1) * VW], wvp[c * 128 : (c + 1) * 128, :])
            nc.gpsimd.dma_start(rows_st[:, 0:VW], bvp[:, :])
            nc.gpsimd.dma_start(rows_st[:, VW:], ones[:, :])
            nc.vector.tensor_copy(wv_b[:], wv_st[:])
            nc.vector.tensor_copy(rows_b[:], rows_st[:])
            for c in range(NCH):
                nc.scalar.dma_start(wq_st[:, c * FPC : (c + 1) * FPC], wq[c * 128 : (c + 1) * 128, :])
            nc.vector.tensor_copy(wq_b[:], wq_st[:])
            nc.scalar.dma_start(bk_sb[:], bk[:, :])
            nc.scalar.dma_start(bq_sb[:], bq[:, :])
            bv_b = rows_b[:, 0:VW]
            ones_b = rows_b[:, VW:]

            # X^T in (t-tile, chunk) slice order: batch 0 fully first.
            for t in range(NTT):
                for c in range(NCH):
                    nc.sync.dma_start(
                        xtc[c][:, t * TT : (t + 1) * TT], xt[c * 128 : (c + 1) * 128, t * TT : (t + 1) * TT]
                    )

            # ---------------- persistent activations -------------------------
            qt_sb = pp.tile([128, NTOK], BF16)   # Q^T: [feat(2 heads), tok]
            kt_sb = pp.tile([128, NTOK], BF16)   # K^T
            vp_sb = pp.tile([128, (NTOK // 128) * VW], BF16)  # V' [tok128, 130] chunks

            # ---------------- projection work units --------------------------
            def qk_tile_units(t, w_b, b_sb, dst):
                """Project t-tile t for Q or K: 8 accumulating matmuls (one
                unit each) + a DVE copy-out with the bias add fused."""
                tsl = slice(t * TT, (t + 1) * TT)
                ps = psP.tile([128, TT], F32, name=f"pj_{t}_{dst.tensor.name}", tag="psP")
                for c in range(NCH):
                    def mm(c=c, ps=ps):
                        nc.tensor.matmul(
                            ps[:], w_b[:, c * FPC : (c + 1) * FPC], xtc[c][:, tsl],
                            start=(c == 0), stop=(c == NCH - 1),
                        )
                    yield 215, mm
                def cp(ps=ps):
                    nc.vector.tensor_scalar_add(dst[:, tsl], ps[:], b_sb[:])
                yield 0, cp

            def v_chunk_units(ch):
                """Project V' for 128-token chunk ch: 9 matmuls + copy,
                split into 3 units for pacing."""
                psv = psP.tile([128, VW], F32, name=f"pv_{ch}", tag="psP", padded_shape=[128, TT])
                def mm_a(psv=psv, ch=ch):
                    for c in range(3):
                        nc.tensor.matmul(
                            psv[:], xtc[c][:, ch * 128 : (ch + 1) * 128],
                            wv_b[:, c * VW : (c + 1) * VW],
                            start=(c == 0), stop=False,
                        )
                def mm_b(psv=psv, ch=ch):
                    for c in range(3, 6):
                        nc.tensor.matmul(
                            psv[:], xtc[c][:, ch * 128 : (ch + 1) * 128],
                            wv_b[:, c * VW : (c + 1) * VW],
                            start=False, stop=False,
                        )
                def mm_c(psv=psv, ch=ch):
                    for c in range(6, NCH):
                        nc.tensor.matmul(
                            psv[:], xtc[c][:, ch * 128 : (ch + 1) * 128],
                            wv_b[:, c * VW : (c + 1) * VW],
                            start=False, stop=False,
                        )
                    nc.tensor.matmul(psv[:], ones_b, bv_b, start=False, stop=True)
                    nc.vector.tensor_copy(vp_sb[:, ch * VW : (ch + 1) * VW], psv[:])
                yield 190, mm_a
                yield 190, mm_b
                yield 190, mm_c

            def proj_units(tiles_qk, v_chunks):
                """Interleave K/Q t-tile units with V'-chunk units."""
                for kind, arg in tiles_qk:
                    if kind == "k":
                        yield from qk_tile_units(arg, wk_b, bk_sb, kt_sb)
                    elif kind == "q":
                        yield from qk_tile_units(arg, wq_b, bq_sb, qt_sb)
                    else:
                        yield from v_chunk_units(arg)

            def emit_all(units):
                for _, fn in units:
                    fn()

            # Work queues: P0 runs before attention; QA during b0 attention
            # (must drain before b1 attention); QB during (b1, qh0).
            P0 = []
            for t in range(4):           # batch-0: K tile then its 4 V' chunks
                P0.append(("k", t))
                for j in range(4):
                    P0.append(("v", t * 4 + j))
            P0 += [("q", 0), ("q", 1)]   # Q for (b0, qh0)

            QA = [("q", 2), ("q", 3)]    # Q for (b0, qh1)
            for t in range(4, 8):        # batch-1 K, V'
                QA.append(("k", t))
                for j in range(4):
                    QA.append(("v", t * 4 + j))
            QA += [("q", 4), ("q", 5)]   # Q for (b1, qh0)

            QB = [("q", 6), ("q", 7)]    # Q for (b1, qh1)

            emit_all(proj_units(P0, None))

            queues = {0: iter(proj_units(QA, None)), 1: iter(proj_units(QB, None))}

            def pull(qi, budget):
                it = queues.get(qi)
                if it is None:
                    return
                spent = 0
                while spent < budget:
                    try:
                        cost, fn = next(it)
                    except StopIteration:
                        queues[qi] = None
                        return
                    fn()
                    spent += max(cost, 1)

            def drain(qi):
                it = queues.get(qi)
                if it is not None:
                    for _, fn in it:
                        fn()
                    queues[qi] = None

            # ---------------- attention --------------------------------------
            def attn_head(b, qh, h, interleave_qi=None):
                """One (batch, q-half, head): 16 k-chunk steps, one-step
                software pipeline (PV of step kt emitted after sim/exp of
                kt+1 so PE never waits on ACT)."""
                hp = h * HS
                pvp = psB.tile([HS + 1, QH], F32, name=f"pvp_{b}_{qh}_{h}", tag="psB")
                pts = {}

                def sim_exp(kt):
                    ksl = b * S + kt * KT
                    sim = psA.tile([128, QH], F32, name=f"sim_{b}_{qh}_{h}_{kt}", tag="psA")
                    for qq in range(2):
                        qsl = b * S + qh * QH + qq * QT
                        nc.tensor.matmul(
                            sim[:, qq * QT : (qq + 1) * QT],
                            kt_sb[hp : hp + HS, ksl : ksl + KT],
                            qt_sb[hp : hp + HS, qsl : qsl + QT],
                            start=True, stop=True,
                        )
                    pt = wk_pool.tile([128, QH], BF16, name=f"pt_{b}_{qh}_{h}_{kt}", tag="pt", bufs=4)
                    nc.scalar.activation(pt[:], sim[:], mybir.ActivationFunctionType.Exp, scale=1.0 / np.sqrt(HS))
                    pts[kt] = pt

                def pv(kt):
                    ch = (b * S) // 128 + kt
                    for qq in range(2):
                        nc.tensor.matmul(
                            pvp[:, qq * QT : (qq + 1) * QT],
                            vp_sb[:, ch * VW + h * (HS + 1) : ch * VW + (h + 1) * (HS + 1)],
                            pts[kt][:, qq * QT : (qq + 1) * QT],
                            start=(kt == 0), stop=(kt == NKT - 1),
                        )
                    del pts[kt]

                sim_exp(0)
                for kt in range(1, NKT):
                    sim_exp(kt)
                    if interleave_qi is not None:
                        pull(interleave_qi, 120)
                    pv(kt - 1)
                pv(NKT - 1)

                ot = wk_pool.tile([HS + 1, QH], F32, name=f"ot_{b}_{qh}_{h}", tag="ot", bufs=2)
                nc.vector.tensor_copy(ot[:], pvp[:])
                nc.sync.dma_start(
                    out[h * (HS + 1) : (h + 1) * (HS + 1), b * S + qh * QH : b * S + (qh + 1) * QH],
                    ot[:],
                )

            for qh in range(2):
                for h in range(2):
                    attn_head(0, qh, h, interleave_qi=0)
            drain(0)
            attn_head(1, 0, 0, interleave_qi=1)
            attn_head(1, 0, 1, interleave_qi=1)
            drain(1)
            attn_head(1, 1, 0)
            attn_head(1, 1, 1)

    nc.compile()
    return nc


def get_nc():
    if "nc" not in _NC_CACHE:
        _NC_CACHE["nc"] = build_nc()
    return _NC_CACHE["nc"]


def make_in_maps(seq_input, WQ, bQ, WK, bK, WV, bV):
    x = np.asarray(seq_input, dtype=np.float32).reshape(NTOK, D)
    xt = np.ascontiguousarray(x.T).astype(ml_dtypes.bfloat16)
    ones = np.ones((1, KT), dtype=np.float32)
    in_maps = []
    for c in range(NCORES):
        lo, hi = c * FPC, (c + 1) * FPC
        wvp = np.zeros((D, VW), dtype=np.float32)
        wvp[:, 0:HS] = WV[:, lo : lo + HS]
        wvp[:, HS + 1 : 2 * HS + 1] = WV[:, lo + HS : hi]
        bvp = np.zeros((1, VW), dtype=np.float32)
        bvp[0, 0:HS] = bV[lo : lo + HS]
        bvp[0, HS] = 1.0
        bvp[0, HS + 1 : 2 * HS + 1] = bV[lo + HS : hi]
        bvp[0, 2 * HS + 1] = 1.0
        in_maps.append(
            {
                "xt": xt,
                "wq": np.ascontiguousarray(WQ[:, lo:hi]),
                "wk": np.ascontiguousarray(WK[:, lo:hi]),
                "wvp": wvp,
                "bq": np.ascontiguousarray(bQ[lo:hi]).reshape(FPC, 1),
                "bk": np.ascontiguousarray(bK[lo:hi]).reshape(FPC, 1),
                "bvp": bvp,
                "ones": ones,
            }
        )
    return in_maps


def run(in_maps, trace=False):
    nc = get_nc()
    return bass_utils.run_bass_kernel_spmd(nc, in_maps, core_ids=list(range(NCORES)), trace=trace)


def kernel(seq_input, WQ, bQ, WK, bK, WV, bV):
    in_maps = make_in_maps(
        np.asarray(seq_input, np.float32),
        np.asarray(WQ, np.float32), np.asarray(bQ, np.float32),
        np.asarray(WK, np.float32), np.asarray(bK, np.float32),
        np.asarray(WV, np.float32), np.asarray(bV, np.float32),
    )
    res = run(in_maps)
    parts = []
    for c in range(NCORES):
        o = res.results[c]["out"]  # [130, 4096] feature-major, unnormalized
        for h in range(2):
            num = o[h * (HS + 1) : h * (HS + 1) + HS, :]      # [64, 4096]
            den = o[h * (HS + 1) + HS, :]                     # [4096]
            parts.append((num / den).T)                       # [4096, 64]
    full = np.concatenate(parts, axis=1)  # [4096, 1024]
    return full.reshape(B, S, H * HS)


# revision 5
# speedup vs baseline: 1.0348x; 1.0348x over previous
"""Multi-head attention Trainium2 Bass kernel.

Problem: B=2, S=2048, D=1024, H=16, HS=64.
Sharding: tensor-parallel over heads — each of 8 cores computes 2 heads
(128 contiguous output-feature columns) for both batches; host concatenates.

Per-core pipeline (v3). Measured facts this schedule is built around:
  - The exp stream on ACT is the steady-state limiter: one [128,1024]
    PSUM->SBUF exp is ~1114 ns, two per k-chunk step -> 2228 ns cadence.
  - PE issues N=512 bf16 matmuls at ~215 ns; a k-chunk step needs 4 sim
    + 4 PV matmuls (~1.8 us incl weight loads) so PE has ~0.4 us slack
    per step. All projection work that can't run before the first
    attention step is emitted INTO this slack (psA/psB pool rotation
    slots between sim tiles).
  - DMA is packetized per row; keep [128, 2048] slabs (4KB rows) and
    order batch 0 first so the first projections start at ~3 us.

Phases:
  1. X^T slabs (b0 then b1) on the sync queue; weights on the scalar and
     gpsimd queues. K t-tiles 0-3 are projected with their 8 contraction
     matmuls interleaved across tiles (2 psA + 2 psB slots) so each
     matmul fires as its X^T chunk lands; then Q t0,t1 and V' chunks 0-2.
     Q/K bias is fused into the PSUM->SBUF copy on DVE (per-partition
     bias column); V' keeps the ones-column bias matmul (N=130).
  2. Attention per (batch, q-half): 16 k-chunk steps of paired-head
     K=64 sims, exp, PV accumulation into per-head [65,1024] PSUM
     (row 65 = softmax denominator from the ones columns in V').
     Remaining projection tiles (Q qh1, all of batch 1) are pulled one
     unit per step from a work queue between the pv emissions.
  3. Unnormalized O'^T (with denominator row) goes to DRAM; the host
     divides and transposes during assembly.
"""

import sys

sys.path.insert(0, "/opt/trn_rl_repo")

import ml_dtypes
import numpy as np

import concourse.bass as bass
import concourse.mybir as mybir
import concourse.tile as tile
from concourse import bacc
from concourse import bass_utils

B, S, D = 2, 2048, 1024
H, HS = 16, 64
NCORES = 8
NTOK = B * S                  # 4096
FPC = (H // NCORES) * HS      # 128 output-feature cols per core (2 heads)
TT = 512                      # token tile for projections
NTT = NTOK // TT              # 8
NCH = D // 128                # 8 contraction chunks
QT = 512                      # q tile (one matmul / psum bank)
QH = 2 * QT                   # 1024-wide q half
KT = 128                      # k chunk in attention
NKT = S // KT                 # 16
VW = 2 * (HS + 1)             # 130: [V_h0 | 1 | V_h1 | 1] columns

F32 = mybir.dt.float32
BF16 = mybir.dt.bfloat16

_NC_CACHE = {}


def build_nc():
    nc = bacc.Bacc("TRN2", target_bir_lowering=False, debug=False, num_devices=NCORES)
    xt = nc.dram_tensor("xt", [D, NTOK], BF16, kind="ExternalInput").ap()
    wq = nc.dram_tensor("wq", [D, FPC], F32, kind="ExternalInput").ap()
    wk = nc.dram_tensor("wk", [D, FPC], F32, kind="ExternalInput").ap()
    wvp = nc.dram_tensor("wvp", [D, VW], F32, kind="ExternalInput").ap()
    bq = nc.dram_tensor("bq", [FPC, 1], F32, kind="ExternalInput").ap()
    bk = nc.dram_tensor("bk", [FPC, 1], F32, kind="ExternalInput").ap()
    bvp = nc.dram_tensor("bvp", [1, VW], F32, kind="ExternalInput").ap()
    ones = nc.dram_tensor("ones", [1, KT], F32, kind="ExternalInput").ap()
    out = nc.dram_tensor("out", [2 * (HS + 1), NTOK], F32, kind="ExternalOutput").ap()

    with tile.TileContext(nc) as tc:
        with (
            tc.tile_pool(name="persist", bufs=1) as pp,
            tc.tile_pool(name="work", bufs=2) as wk_pool,
            tc.tile_pool(name="psA", bufs=2, space="PSUM") as psA,
            tc.tile_pool(name="psB", bufs=2, space="PSUM") as psB,
        ):
            # ---------------- init: weights, X^T ------------------------------
            wq_st = pp.tile([128, NCH * FPC], F32)
            wk_st = pp.tile([128, NCH * FPC], F32)
            wv_st = pp.tile([128, NCH * VW], F32)
            xtc = [pp.tile([128, NTOK], BF16, name=f"xt_{c}") for c in range(NCH)]
            wq_b = pp.tile([128, NCH * FPC], BF16)
            wk_b = pp.tile([128, NCH * FPC], BF16)
            wv_b = pp.tile([128, NCH * VW], BF16)
            bq_sb = pp.tile([128, 1], F32)
            bk_sb = pp.tile([128, 1], F32)
            rows_st = pp.tile([1, VW + KT], F32)
            rows_b = pp.tile([1, VW + KT], BF16)

            # K weights first (scalar HWDGE queue) so the first projection's
            # weights beat its activations; V'/Q weights on gpsimd.
            for c in range(NCH):
                nc.scalar.dma_start(wk_st[:, c * FPC : (c + 1) * FPC], wk[c * 128 : (c + 1) * 128, :])
            nc.vector.tensor_copy(wk_b[:], wk_st[:])
            nc.scalar.dma_start(bk_sb[:], bk[:, :])
            nc.scalar.dma_start(bq_sb[:], bq[:, :])
            for c in range(NCH):
                nc.gpsimd.dma_start(wv_st[:, c * VW : (c + 1) * VW], wvp[c * 128 : (c + 1) * 128, :])
            nc.gpsimd.dma_start(rows_st[:, 0:VW], bvp[:, :])
            nc.gpsimd.dma_start(rows_st[:, VW:], ones[:, :])
            nc.vector.tensor_copy(wv_b[:], wv_st[:])
            nc.vector.tensor_copy(rows_b[:], rows_st[:])
            for c in range(NCH):
                nc.gpsimd.dma_start(wq_st[:, c * FPC : (c + 1) * FPC], wq[c * 128 : (c + 1) * 128, :])
            nc.vector.tensor_copy(wq_b[:], wq_st[:])
            bv_b = rows_b[:, 0:VW]
            ones_b = rows_b[:, VW:]

            # X^T slabs: 4KB rows, batch 0 fully first.
            for c in range(NCH):
                nc.sync.dma_start(xtc[c][:, 0:S], xt[c * 128 : (c + 1) * 128, 0:S])
            for c in range(NCH):
                nc.sync.dma_start(xtc[c][:, S : 2 * S], xt[c * 128 : (c + 1) * 128, S : 2 * S])

            # ---------------- persistent activations -------------------------
            qt_sb = pp.tile([128, NTOK], BF16)   # Q^T: [feat(2 heads), tok]
            kt_sb = pp.tile([128, NTOK], BF16)   # K^T
            vp_sb = pp.tile([128, (NTOK // 128) * VW], BF16)  # V' [tok128, 130] chunks

            # ---------------- projection emitters -----------------------------
            def qk_tile(t, w_b, b_sb, dst, pool):
                """One Q/K t-tile: psum tile + 8 matmuls; DVE copy with fused
                bias. Returns emitters so P0 can interleave chunk matmuls."""
                tsl = slice(t * TT, (t + 1) * TT)
                ps = pool.tile([128, TT], F32, name=f"pj_{t}_{dst.tensor.name}",
                               tag=pool.name, padded_shape=[128, QH])
                def mm(c):
                    nc.tensor.matmul(
                        ps[:], w_b[:, c * FPC : (c + 1) * FPC], xtc[c][:, tsl],
                        start=(c == 0), stop=(c == NCH - 1),
                    )
                def cp():
                    nc.vector.tensor_scalar_add(dst[:, tsl], ps[:], b_sb[:])
                return mm, cp

            def emit_qk_tile(t, w_b, b_sb, dst, pool):
                mm, cp = qk_tile(t, w_b, b_sb, dst, pool)
                for c in range(NCH):
                    mm(c)
                cp()

            def emit_v_chunk(ch, pool):
                psv = pool.tile([128, VW], F32, name=f"pv_{ch}", tag=pool.name,
                                padded_shape=[128, QH])
                for c in range(NCH):
                    nc.tensor.matmul(
                        psv[:], xtc[c][:, ch * 128 : (ch + 1) * 128],
                        wv_b[:, c * VW : (c + 1) * VW],
                        start=(c == 0), stop=False,
                    )
                nc.tensor.matmul(psv[:], ones_b, bv_b, start=False, stop=True)
                nc.vector.tensor_copy(vp_sb[:, ch * VW : (ch + 1) * VW], psv[:])

            # ---------------- P0: batch-0 K, Q(qh0), first V' chunks ----------
            # K tiles 0-3 with contraction matmuls interleaved across tiles so
            # each fires as its X^T chunk arrives (2 psA slots + 2 psB slots).
            k_mms = []
            for t, pool in ((0, psA), (1, psA), (2, psB), (3, psB)):
                k_mms.append(qk_tile(t, wk_b, bk_sb, kt_sb, pool))
            for c in range(NCH):
                for mm, _ in k_mms:
                    mm(c)
            for _, cp in k_mms:
                cp()
            emit_qk_tile(0, wq_b, bq_sb, qt_sb, psA)
            emit_qk_tile(1, wq_b, bq_sb, qt_sb, psA)
            for ch in (0, 1, 2):
                emit_v_chunk(ch, psB)

            # ---------------- interleave work queues --------------------------
            def unit_qk(t, w_b, b_sb, dst):
                def f():
                    emit_qk_tile(t, w_b, b_sb, dst, psA)
                return 1900, f

            def unit_v(ch):
                def f():
                    emit_v_chunk(ch, psA)
                return 600, f

            QA = [unit_v(ch) for ch in range(3, 16)]
            QA += [unit_qk(2, wq_b, bq_sb, qt_sb), unit_qk(3, wq_b, bq_sb, qt_sb)]
            QA += [unit_qk(t, wk_b, bk_sb, kt_sb) for t in range(4, 8)]
            QA += [unit_qk(4, wq_b, bq_sb, qt_sb), unit_qk(5, wq_b, bq_sb, qt_sb)]
            QA += [unit_v(ch) for ch in range(16, 20)]
            QB = [unit_v(ch) for ch in range(20, 32)]
            QB += [unit_qk(6, wq_b, bq_sb, qt_sb), unit_qk(7, wq_b, bq_sb, qt_sb)]
            queues = {0: iter(QA), 1: iter(QB)}

            def pull(qi, budget):
                it = queues.get(qi)
                if it is None:
                    return
                spent = 0
                while spent < budget:
                    try:
                        cost, fn = next(it)
                    except StopIteration:
                        queues[qi] = None
                        return
                    fn()
                    spent += cost

            def drain(qi):
                it = queues.get(qi)
                if it is not None:
                    for _, fn in it:
                        fn()
                    queues[qi] = None

            # ---------------- attention --------------------------------------
            def attn_phase(b, qh, interleave_qi=None):
                pvp = [
                    psB.tile([HS + 1, QH], F32, name=f"pvp_{b}_{qh}_{h}", tag="psB",
                             padded_shape=[128, QH])
                    for h in range(2)
                ]
                for kt in range(NKT):
                    ksl = b * S + kt * KT
                    ch = (b * S) // 128 + kt
                    sims = [
                        psA.tile([128, QH], F32, name=f"sim_{b}_{qh}_{kt}_{h}", tag="psA",
                                 padded_shape=[128, QH])
                        for h in range(2)
                    ]
                    # alternate heads so the K=64 matmuls pack into disjoint
                    # PE row groups (h0 rows 0-63, h1 rows 64-127)
                    for qq in range(2):
                        for h in range(2):
                            hp = h * HS
                            qsl = b * S + qh * QH + qq * QT
                            nc.tensor.matmul(
                                sims[h][:, qq * QT : (qq + 1) * QT],
                                kt_sb[hp : hp + HS, ksl : ksl + KT],
                                qt_sb[hp : hp + HS, qsl : qsl + QT],
                                start=True, stop=True,
                                tile_position=(hp, 0),
                            )
                    pts = []
                    for h in range(2):
                        pt = wk_pool.tile([128, QH], BF16, name=f"pt_{b}_{qh}_{kt}_{h}", tag="pt", bufs=6)
                        nc.scalar.activation(pt[:], sims[h][:], mybir.ActivationFunctionType.Exp, scale=1.0 / np.sqrt(HS))
                        pts.append(pt)
                    if interleave_qi is not None:
                        pull(interleave_qi, 500)
                    for h in range(2):
                        for qq in range(2):
                            nc.tensor.matmul(
                                pvp[h][:, qq * QT : (qq + 1) * QT],
                                vp_sb[:, ch * VW + h * (HS + 1) : ch * VW + (h + 1) * (HS + 1)],
                                pts[h][:, qq * QT : (qq + 1) * QT],
                                start=(kt == 0), stop=(kt == NKT - 1),
                            )
                for h in range(2):
                    ot = wk_pool.tile([HS + 1, QH], F32, name=f"ot_{b}_{qh}_{h}", tag="ot", bufs=2)
                    nc.vector.tensor_copy(ot[:], pvp[h][:])
                    nc.sync.dma_start(
                        out[h * (HS + 1) : (h + 1) * (HS + 1), b * S + qh * QH : b * S + (qh + 1) * QH],
                        ot[:],
                    )

            attn_phase(0, 0, interleave_qi=0)
            attn_phase(0, 1, interleave_qi=0)
            drain(0)
            attn_phase(1, 0, interleave_qi=1)
            drain(1)
            attn_phase(1, 1)

    nc.compile()
    return nc


def get_nc():
    if "nc" not in _NC_CACHE:
        _NC_CACHE["nc"] = build_nc()
    return _NC_CACHE["nc"]


def make_in_maps(seq_input, WQ, bQ, WK, bK, WV, bV):
    x = np.asarray(seq_input, dtype=np.float32).reshape(NTOK, D)
    xt = np.ascontiguousarray(x.T).astype(ml_dtypes.bfloat16)
    ones = np.ones((1, KT), dtype=np.float32)
    in_maps = []
    for c in range(NCORES):
        lo, hi = c * FPC, (c + 1) * FPC
        wvp = np.zeros((D, VW), dtype=np.float32)
        wvp[:, 0:HS] = WV[:, lo : lo + HS]
        wvp[:, HS + 1 : 2 * HS + 1] = WV[:, lo + HS : hi]
        bvp = np.zeros((1, VW), dtype=np.float32)
        bvp[0, 0:HS] = bV[lo : lo + HS]
        bvp[0, HS] = 1.0
        bvp[0, HS + 1 : 2 * HS + 1] = bV[lo + HS : hi]
        bvp[0, 2 * HS + 1] = 1.0
        in_maps.append(
            {
                "xt": xt,
                "wq": np.ascontiguousarray(WQ[:, lo:hi]),
                "wk": np.ascontiguousarray(WK[:, lo:hi]),
                "wvp": wvp,
                "bq": np.ascontiguousarray(bQ[lo:hi]).reshape(FPC, 1),
                "bk": np.ascontiguousarray(bK[lo:hi]).reshape(FPC, 1),
                "bvp": bvp,
                "ones": ones,
            }
        )
    return in_maps


def run(in_maps, trace=False):
    nc = get_nc()
    return bass_utils.run_bass_kernel_spmd(nc, in_maps, core_ids=list(range(NCORES)), trace=trace)


def kernel(seq_input, WQ, bQ, WK, bK, WV, bV):
    in_maps = make_in_maps(
        np.asarray(seq_input, np.float32),
        np.asarray(WQ, np.float32), np.asarray(bQ, np.float32),
        np.asarray(WK, np.float32), np.asarray(bK, np.float32),
        np.asarray(WV, np.float32), np.asarray(bV, np.float32),
    )
    res = run(in_maps)
    parts = []
    for c in range(NCORES):
        o = res.results[c]["out"]  # [130, 4096] feature-major, unnormalized
        for h in range(2):
            num = o[h * (HS + 1) : h * (HS + 1) + HS, :]      # [64, 4096]
            den = o[h * (HS + 1) + HS, :]                     # [4096]
            parts.append((num / den).T)                       # [4096, 64]
    full = np.concatenate(parts, axis=1)  # [4096, 1024]
    return full.reshape(B, S, H * HS)


# revision 7
# speedup vs baseline: 1.2306x; 1.1892x over previous
"""Multi-head attention Trainium2 Bass kernel.

Problem: B=2, S=2048, D=1024, H=16, HS=64.
Sharding: tensor-parallel over heads — each of 8 cores computes 2 heads
(128 contiguous output-feature columns) for both batches; host concatenates.

v4 schedule, built around measured hardware behavior:
  - ACT exp stream: ~1114 ns per [128,1024] PSUM->SBUF tile, 2 per k-step
    (2228 ns cadence) — the attention-phase limiter.
  - PE issues N=512 bf16 matmuls at ~215 ns; per k-step attention needs
    ~1.8 us (4 sim + 4 PV), leaving ~0.4 us slack per step for projection
    work. Total PE work (proj ~47us + attn ~120us) exceeds total ACT work
    (~143us), so the goal is a gap-free PE stream.
  - DMA is packetized per destination row across 16 engines; small rows
    halve throughput. X^T rides [128,1024]/[128,2048] slabs ordered by
    first use; weights are host-packed (bf16, SBUF layout) into one
    [128,3088] tensor so they land in ~2 us on the scalar queue.

Timeline: warmup matmuls ramp the PE p-state and a dummy exp preloads the
ACT table during the DMA lead-in; K/Q projections for (b0, qh0) run with
their contraction matmuls interleaved so each fires as its X^T chunk
arrives; attention starts ~16 us in. All remaining projection tiles are
injected into the attention k-loops in pairs (two psA-pool tiles per
injection keeps the sim double-buffer rotation parity stable), placed so
each lands after its DMA dependency and before its consumer phase. Q/K
bias adds are fused into PSUM->SBUF copies (scalar engine in the prologue,
DVE during attention); split projection tiles merge halves with one
scalar_tensor_tensor. V' keeps the [V_h0|1|V_h1|1] ones-column layout so
PV accumulation yields the softmax denominator as row 65; the host
divides and transposes during assembly.
"""

import sys

sys.path.insert(0, "/opt/trn_rl_repo")

import ml_dtypes
import numpy as np

import concourse.bass as bass
import concourse.mybir as mybir
import concourse.tile as tile
from concourse import bacc
from concourse import bass_utils

B, S, D = 2, 2048, 1024
H, HS = 16, 64
NCORES = 8
NTOK = B * S                  # 4096
FPC = (H // NCORES) * HS      # 128 output-feature cols per core (2 heads)
TT = 512                      # token tile for projections
NTT = NTOK // TT              # 8
NCH = D // 128                # 8 contraction chunks
QT = 512                      # q tile (one matmul / psum bank)
QH = 2 * QT                   # 1024-wide q half
KT = 128                      # k chunk in attention
NKT = S // KT                 # 16
VW = 2 * (HS + 1)             # 130: [V_h0 | 1 | V_h1 | 1] columns
WALLW = 2 * NCH * FPC + NCH * VW   # 3088 packed weight cols

F32 = mybir.dt.float32
BF16 = mybir.dt.bfloat16

_NC_CACHE = {}


def build_nc():
    nc = bacc.Bacc("TRN2", target_bir_lowering=False, debug=False, num_devices=NCORES)
    xt = nc.dram_tensor("xt", [D, NTOK], BF16, kind="ExternalInput").ap()
    wall = nc.dram_tensor("wall", [128, WALLW], BF16, kind="ExternalInput").ap()
    bcol = nc.dram_tensor("bcol", [FPC, 2], F32, kind="ExternalInput").ap()
    brow = nc.dram_tensor("brow", [1, VW + KT], BF16, kind="ExternalInput").ap()
    out = nc.dram_tensor("out", [2 * (HS + 1), NTOK], F32, kind="ExternalOutput").ap()

    with tile.TileContext(nc) as tc:
        with (
            tc.tile_pool(name="persist", bufs=1) as pp,
            tc.tile_pool(name="work", bufs=2) as wk_pool,
            tc.tile_pool(name="psA", bufs=2, space="PSUM") as psA,
            tc.tile_pool(name="psB", bufs=2, space="PSUM") as psB,
        ):
            wall_sb = pp.tile([128, WALLW], BF16)
            bcol_sb = pp.tile([128, 2], F32)
            brow_sb = pp.tile([1, VW + KT], BF16)
            warm_sb = pp.tile([128, TT], BF16)
            junk_sb = pp.tile([128, TT], BF16)
            xtc = [pp.tile([128, NTOK], BF16, name=f"xt_{c}") for c in range(NCH)]
            qt_sb = pp.tile([128, NTOK], BF16)   # Q^T: [feat(2 heads), tok]
            kt_sb = pp.tile([128, NTOK], BF16)   # K^T
            vp_sb = pp.tile([128, (NTOK // 128) * VW], BF16)  # V' [tok128,130] chunks

            def wk_c(c):
                return wall_sb[:, c * FPC : (c + 1) * FPC]

            def wq_c(c):
                return wall_sb[:, NCH * FPC + c * FPC : NCH * FPC + (c + 1) * FPC]

            def wv_c(c):
                o = 2 * NCH * FPC
                return wall_sb[:, o + c * VW : o + (c + 1) * VW]

            bq_ap = bcol_sb[:, 0:1]
            bk_ap = bcol_sb[:, 1:2]
            bv_b = brow_sb[:, 0:VW]
            ones_b = brow_sb[:, VW:]

            # ---------------- DMAs -------------------------------------------
            nc.scalar.dma_start(wall_sb[:], wall[:, :])
            nc.scalar.dma_start(bcol_sb[:], bcol[:, :])
            nc.scalar.dma_start(brow_sb[:], brow[:, :])
            # X^T need-ordered: b0 as 2KB-row halves, b1 as 4KB-row slabs.
            for c in range(NCH):
                nc.sync.dma_start(xtc[c][:, 0:QH], xt[c * 128 : (c + 1) * 128, 0:QH])
            for c in range(NCH):
                nc.sync.dma_start(xtc[c][:, QH:S], xt[c * 128 : (c + 1) * 128, QH:S])
            for c in range(NCH):
                nc.sync.dma_start(xtc[c][:, S : 2 * S], xt[c * 128 : (c + 1) * 128, S : 2 * S])

            # ---------------- warmup (PE p-state ramp + ACT exp table) -------
            nc.gpsimd.memset(warm_sb[:], 0.0)
            warm_ps = psA.tile([128, TT], F32, name="warm", tag="psA", padded_shape=[128, QH])
            for _ in range(5):
                nc.tensor.matmul(warm_ps[:], warm_sb[:, 0:128], warm_sb[:], start=True, stop=True)
            nc.scalar.activation(junk_sb[:], warm_ps[:], mybir.ActivationFunctionType.Exp)

            # ---------------- projection pieces ------------------------------
            def qk_mms(t, wsel, pool, c_lo=0, c_hi=NCH):
                tsl = slice(t * TT, (t + 1) * TT)
                ps = pool.tile([128, TT], F32, name=f"pj{t}_{c_lo}_{wsel.__name__}",
                               tag=pool.name, padded_shape=[128, QH])
                def mm(c):
                    nc.tensor.matmul(ps[:], wsel(c), xtc[c][:, tsl],
                                     start=(c == c_lo), stop=(c == c_hi - 1))
                return ps, mm

            def emit_v_chunk(ch, pool):
                psv = pool.tile([128, VW], F32, name=f"pv_{ch}", tag=pool.name,
                                padded_shape=[128, QH])
                for c in range(NCH):
                    nc.tensor.matmul(
                        psv[:], xtc[c][:, ch * 128 : (ch + 1) * 128], wv_c(c),
                        start=(c == 0), stop=False,
                    )
                nc.tensor.matmul(psv[:], ones_b, bv_b, start=False, stop=True)
                nc.vector.tensor_copy(vp_sb[:, ch * VW : (ch + 1) * VW], psv[:])

            # ---------------- P0: b0 K t0-3 + Q t0,t1 c-interleaved ----------
            p0 = []
            for t, wsel, pool in ((0, wk_c, psA), (1, wk_c, psA),
                                  (0, wq_c, psB), (1, wq_c, psB)):
                p0.append(qk_mms(t, wsel, pool) + (t, wsel))
            for c in range(NCH):
                for ps, mm, t, wsel in p0:
                    mm(c)
            for i, (ps, mm, t, wsel) in enumerate(p0):
                tsl = slice(t * TT, (t + 1) * TT)
                if wsel is wk_c:
                    nc.scalar.activation(kt_sb[:, tsl], ps[:],
                                         mybir.ActivationFunctionType.Identity, bias=bk_ap)
                else:
                    nc.vector.tensor_scalar_add(qt_sb[:, tsl], ps[:], bq_ap)
            for ch in (0, 1, 2, 3):
                emit_v_chunk(ch, psB)

            # ---------------- injected units (each allocates ONE psA tile) ---
            stages = {}

            def v_unit(ch):
                def f():
                    emit_v_chunk(ch, psA)
                return f

            def qk_half(t, wsel, bias_ap, dst, half):
                def f():
                    c_lo, c_hi = half * 4, half * 4 + 4
                    ps, mm = qk_mms(t, wsel, psA, c_lo, c_hi)
                    for c in range(c_lo, c_hi):
                        mm(c)
                    tsl = slice(t * TT, (t + 1) * TT)
                    if half == 0:
                        stg = wk_pool.tile([128, TT], F32, name=f"stg_{t}_{wsel.__name__}",
                                           tag="stg", bufs=2)
                        stages[(t, wsel.__name__)] = stg
                        nc.vector.tensor_scalar_add(stg[:], ps[:], bias_ap)
                    else:
                        stg = stages.pop((t, wsel.__name__))
                        nc.vector.tensor_tensor(dst[:, tsl], stg[:], ps[:],
                                                mybir.AluOpType.add)
                return f

            def K_(t, half):
                return qk_half(t, wk_c, bk_ap, kt_sb, half)

            def Q_(t, half):
                return qk_half(t, wq_c, bq_ap, qt_sb, half)

            SCHED = {
                (0, 0): {
                    0: [v_unit(4), v_unit(5)], 1: [v_unit(6), v_unit(7)],
                    4: [K_(2, 0), K_(2, 1)], 5: [K_(3, 0), K_(3, 1)],
                    6: [v_unit(8), v_unit(9)], 7: [v_unit(10), v_unit(11)],
                    8: [v_unit(12), v_unit(13)], 9: [v_unit(14), v_unit(15)],
                    10: [Q_(2, 0), Q_(2, 1)], 11: [Q_(3, 0), Q_(3, 1)],
                    14: [K_(4, 0), K_(4, 1)], 15: [K_(5, 0), K_(5, 1)],
                },
                (0, 1): {
                    0: [K_(6, 0), K_(6, 1)], 1: [K_(7, 0), K_(7, 1)],
                    2: [Q_(4, 0), Q_(4, 1)], 3: [Q_(5, 0), Q_(5, 1)],
                },
                (1, 0): {
                    0: [v_unit(16), v_unit(17)], 1: [v_unit(18), v_unit(19)],
                    2: [v_unit(20), v_unit(21)], 3: [v_unit(22), v_unit(23)],
                    4: [v_unit(24), v_unit(25)], 5: [v_unit(26), v_unit(27)],
                    6: [v_unit(28), v_unit(29)], 7: [v_unit(30), v_unit(31)],
                    8: [Q_(6, 0), Q_(6, 1)], 9: [Q_(7, 0), Q_(7, 1)],
                },
                (1, 1): {},
            }

            # ---------------- attention --------------------------------------
            def attn_phase(b, qh):
                sched = SCHED[(b, qh)]
                pvp = [
                    psB.tile([HS + 1, QH], F32, name=f"pvp_{b}_{qh}_{h}", tag="psB",
                             padded_shape=[128, QH])
                    for h in range(2)
                ]
                pts = {}

                def pvs(kt):
                    ch = (b * S) // 128 + kt
                    for h in range(2):
                        for qq in range(2):
                            nc.tensor.matmul(
                                pvp[h][:, qq * QT : (qq + 1) * QT],
                                vp_sb[:, ch * VW + h * (HS + 1) : ch * VW + (h + 1) * (HS + 1)],
                                pts[kt][h][:, qq * QT : (qq + 1) * QT],
                                start=(kt == 0), stop=(kt == NKT - 1),
                            )
                    del pts[kt]

                for kt in range(NKT):
                    ksl = b * S + kt * KT
                    sims = [
                        psA.tile([128, QH], F32, name=f"sim_{b}_{qh}_{kt}_{h}", tag="psA",
                                 padded_shape=[128, QH])
                        for h in range(2)
                    ]
                    for qq in range(2):
                        for h in range(2):
                            hp = h * HS
                            qsl = b * S + qh * QH + qq * QT
                            nc.tensor.matmul(
                                sims[h][:, qq * QT : (qq + 1) * QT],
                                kt_sb[hp : hp + HS, ksl : ksl + KT],
                                qt_sb[hp : hp + HS, qsl : qsl + QT],
                                start=True, stop=True,
                                tile_position=(hp, 0),
                            )
                    cur = []
                    for h in range(2):
                        pt = wk_pool.tile([128, QH], BF16, name=f"pt_{b}_{qh}_{kt}_{h}",
                                          tag="pt", bufs=6)
                        nc.scalar.activation(pt[:], sims[h][:],
                                             mybir.ActivationFunctionType.Exp,
                                             scale=1.0 / np.sqrt(HS))
                        cur.append(pt)
                    pts[kt] = cur
                    if kt > 0:
                        pvs(kt - 1)
                    for fn in sched.get(kt, ()):
                        fn()
                pvs(NKT - 1)
                for h in range(2):
                    ot = wk_pool.tile([HS + 1, QH], F32, name=f"ot_{b}_{qh}_{h}",
                                      tag="ot", bufs=2)
                    nc.vector.tensor_copy(ot[:], pvp[h][:])
                    nc.sync.dma_start(
                        out[h * (HS + 1) : (h + 1) * (HS + 1), b * S + qh * QH : b * S + (qh + 1) * QH],
                        ot[:],
                    )

            attn_phase(0, 0)
            attn_phase(0, 1)
            attn_phase(1, 0)
            attn_phase(1, 1)

    nc.compile()
    return nc


def get_nc():
    if "nc" not in _NC_CACHE:
        _NC_CACHE["nc"] = build_nc()
    return _NC_CACHE["nc"]


def make_in_maps(seq_input, WQ, bQ, WK, bK, WV, bV):
    x = np.asarray(seq_input, dtype=np.float32).reshape(NTOK, D)
    xt = np.ascontiguousarray(x.T).astype(ml_dtypes.bfloat16)
    in_maps = []
    for c in range(NCORES):
        lo, hi = c * FPC, (c + 1) * FPC
        wall = np.zeros((128, WALLW), dtype=np.float32)
        for ch in range(NCH):
            rs = slice(ch * 128, (ch + 1) * 128)
            wall[:, ch * FPC : (ch + 1) * FPC] = WK[rs, lo:hi]
            wall[:, NCH * FPC + ch * FPC : NCH * FPC + (ch + 1) * FPC] = WQ[rs, lo:hi]
            o = 2 * NCH * FPC
            wall[:, o + ch * VW : o + ch * VW + HS] = WV[rs, lo : lo + HS]
            wall[:, o + ch * VW + HS + 1 : o + ch * VW + 2 * HS + 1] = WV[rs, lo + HS : hi]
        bcol = np.stack([bQ[lo:hi], bK[lo:hi]], axis=1).astype(np.float32)
        brow = np.zeros((1, VW + KT), dtype=np.float32)
        brow[0, 0:HS] = bV[lo : lo + HS]
        brow[0, HS] = 1.0
        brow[0, HS + 1 : 2 * HS + 1] = bV[lo + HS : hi]
        brow[0, 2 * HS + 1] = 1.0
        brow[0, VW:] = 1.0
        in_maps.append(
            {
                "xt": xt,
                "wall": wall.astype(ml_dtypes.bfloat16),
                "bcol": np.ascontiguousarray(bcol),
                "brow": brow.astype(ml_dtypes.bfloat16),
            }
        )
    return in_maps


def run(in_maps, trace=False):
    nc = get_nc()
    return bass_utils.run_bass_kernel_spmd(nc, in_maps, core_ids=list(range(NCORES)), trace=trace)


def kernel(seq_input, WQ, bQ, WK, bK, WV, bV):
    in_maps = make_in_maps(
        np.asarray(seq_input, np.float32),
        np.asarray(WQ, np.float32), np.asarray(bQ, np.float32),
        np.asarray(WK, np.float32), np.asarray(bK, np.float32),
        np.asarray(WV, np.float32), np.asarray(bV, np.float32),
    )
    res = run(in_maps)
    parts = []
    for c in range(NCORES):
        o = res.results[c]["out"]  # [130, 4096] feature-major, unnormalized
        for h in range(2):
            num = o[h * (HS + 1) : h * (HS + 1) + HS, :]      # [64, 4096]
            den = o[h * (HS + 1) + HS, :]                     # [4096]
            parts.append((num / den).T)                       # [4096, 64]
    full = np.concatenate(parts, axis=1)  # [4096, 1024]
    return full.reshape(B, S, H * HS)
